# revision 2
# baseline (speedup 1.0000x reference)
"""Trainium2 Bass kernel v2 for nn_BasicGRUBlock: 2-layer GRU block.

  x = y + z; h1 = GRU0(x); h2 = GRU1(h1); out = y + h2 @ W_lin.T + b_lin

Sharding: data-parallel over batch across 8 cores (8 sequences/core).

v2 design: fully-transposed recurrence (hidden dim on 128 partitions,
batch in the free dim).  Per 8-step group and layer, one 2KB PSUM bank
holds [128, 8mc, 8t, 8b]: mc 0-3 = rz gates (gx+biases preloaded by the
bulk matmuls, gh accumulated per step on top), mc 4-5 = gh_n (+b_hh_n
preload), mc 6-7 = gx_n (+b_ih_n).  A GRU step is 12 weight-stationary
128x128 matmuls + a 7-hop serial chain:
  PE(gh) -> ACT(sig r) -> DVE(m=r*ghn) -> Pool(tn=m+gxn) -> ACT(tanh)
  -> Pool(p=(1-z)*n) -> DVE(h'=p+z*h)
with sigmoid(z), w=1-z, a=z*h off the critical path.  No per-step
transposes, no reshape DMAs.  L0 (group s) and L1 (group s-1) steps are
emitted interleaved so the two chains overlap on the in-order engines.

Super-iteration s: L0 steps group s, L1 steps group s-1; then bulk:
gx1 matmuls for group s (feeds L1 next super), final linear + residual
+ store for group s-1, x-prep + gx0 matmuls for group s+1, y/z loads
for group s+2.  Supers 0,1 and SUP-2..SUP are Python-emitted
(pipeline fill/drain guards); supers 2..SUP-3 run in a For_i hw loop.
"""

import sys

sys.path.insert(0, "/opt/trn_rl_repo")

import numpy as np

import concourse.bass as bass
import concourse.bacc as bacc_mod
import concourse.mybir as mybir
from concourse.bass import ds
from concourse.tile import TileContext

B, T_FULL, I, H, G = 64, 4096, 64, 256, 768
NCORES = 8
BL = B // NCORES  # 8 sequences per core
GRP = 8  # time steps per group
F32 = mybir.dt.float32
F32R = mybir.dt.float32r
FP8E4 = mybir.dt.float8e4
DR = mybir.MatmulPerfMode.DoubleRow

SIG = mybir.ActivationFunctionType.Sigmoid
TANH = mybir.ActivationFunctionType.Tanh
MULT = mybir.AluOpType.mult
ADD = mybir.AluOpType.add
SUB = mybir.AluOpType.subtract


STAGGERED = True
STAGE_BOUNDARIES = True


RZ_FOLD_PE = False
FP8 = True
FP8_MIXED = True
Q_INPLACE = True


def _r(ap):
    return ap.bitcast(F32R)


def build_nc(T=T_FULL, unroll=4, use_loop=True, debug=False, repeats=1):
    nc = bacc_mod.Bacc()
    dbg = {}
    if debug:
        for nm in ("gxn0", "h0g", "gxn1", "h1g", "ps0", "ps1"):
            dbg[nm] = nc.declare_dram_parameter(
                f"dbg_{nm}", [128, 6 if nm.startswith("ps") else 2, GRP, BL],
                F32, isOutput=True)

    y_d = nc.declare_dram_parameter("y", [BL, T, I], F32, isOutput=False)
    z_d = nc.declare_dram_parameter("z", [BL, T, I], F32, isOutput=False)
    w0T_d = nc.declare_dram_parameter("w0T", [I + 1, 6, 128], F32R, isOutput=False)
    WDT = FP8E4 if FP8 else F32R
    whh0T_d = nc.declare_dram_parameter("whh0T", [128, 2, 6, 128], WDT, isOutput=False)
    bhh0n_d = nc.declare_dram_parameter("bhh0n", [128, 2], F32, isOutput=False)
    w1T_d = nc.declare_dram_parameter("w1T", [128, 2, 6, 128], WDT, isOutput=False)
    whh1T_d = nc.declare_dram_parameter("whh1T", [128, 2, 6, 128], WDT, isOutput=False)
    b1bc_d = nc.declare_dram_parameter("b1bc", [128, 6, GRP, BL], F32, isOutput=False)
    eye128_d = nc.declare_dram_parameter("eye128", [128, 128], F32R, isOutput=False)
    bhh1n_d = nc.declare_dram_parameter("bhh1n", [128, 2], F32, isOutput=False)
    LDT = F32R if FP8_MIXED else WDT
    wlinT_d = nc.declare_dram_parameter("wlinT", [128, 2, I], LDT, isOutput=False)
    whh0n_d = nc.declare_dram_parameter("whh0n", [128, 2, 2, 128], F32R, isOutput=False)
    bhh0nr_d = nc.declare_dram_parameter("bhh0nr", [1, 2, 128], F32R, isOutput=False)
    bhh1nr_d = nc.declare_dram_parameter("bhh1nr", [1, 2, 128], F32R, isOutput=False)
    whh1n_d = nc.declare_dram_parameter("whh1n", [128, 2, 2, 128], F32R, isOutput=False)
    w1n_d = nc.declare_dram_parameter("w1n", [128, 2, 2, 128], F32R, isOutput=False)
    blinbc_d = nc.declare_dram_parameter("blinbc", [64, I], F32, isOutput=False)
    eye_d = nc.declare_dram_parameter("eye64", [64, 64], F32, isOutput=False)
    out_d = nc.declare_dram_parameter("out", [BL, T, I], F32, isOutput=True)

    assert T % GRP == 0
    SUP = T // GRP

    with TileContext(nc) as tc:
        with (
            tc.tile_pool(name="wpool", bufs=1) as wpool,
            tc.tile_pool(name="iopool", bufs=4) as iopool,
            tc.tile_pool(name="opool", bufs=2) as opool,
            tc.tile_pool(name="gpool", bufs=4) as gpool,
            tc.tile_pool(name="ps_gate", bufs=1, space="PSUM") as ps_gate,
            tc.tile_pool(name="ps_misc", bufs=2, space="PSUM") as ps_misc,
        ):
            # ---- persistent weights / constants ----
            w0T_t = wpool.tile([I + 1, 6, 128], F32R)
            whh0T_t = wpool.tile([128, 2, 6, 128], WDT)
            bhh0n_t = wpool.tile([128, 2], F32)
            w1T_t = wpool.tile([128, 2, 6, 128], WDT)
            whh1T_t = wpool.tile([128, 2, 6, 128], WDT)
            b1bc_t = wpool.tile([128, 6, GRP, BL], F32)
            eye128_t = wpool.tile([128, 128], F32R)
            bhh1n_t = wpool.tile([128, 2], F32)
            wlinT_t = wpool.tile([128, 2, I], LDT)
            whh0n_t = wpool.tile([128, 2, 2, 128], F32R)
            bhh0nr_t = wpool.tile([1, 2, 128], F32R)
            bhh1nr_t = wpool.tile([1, 2, 128], F32R)
            whh1n_t = wpool.tile([128, 2, 2, 128], F32R)
            w1n_t = wpool.tile([128, 2, 2, 128], F32R)
            blinbc_t = wpool.tile([64, I], F32)
            eye_t = wpool.tile([64, 64], F32)
            onesf_t = wpool.tile([1, 64], F32)

            for t, d in ((w0T_t, w0T_d), (whh0T_t, whh0T_d), (bhh0n_t, bhh0n_d),
                         (w1T_t, w1T_d), (whh1T_t, whh1T_d), (b1bc_t, b1bc_d),
                         (eye128_t, eye128_d), (bhh1n_t, bhh1n_d), (wlinT_t, wlinT_d),
                         (whh0n_t, whh0n_d), (whh1n_t, whh1n_d), (w1n_t, w1n_d),
                         (bhh0nr_t, bhh0nr_d), (bhh1nr_t, bhh1nr_d),
                         (blinbc_t, blinbc_d), (eye_t, eye_d)):
                nc.sync.dma_start(out=t, in_=d[:])
            ones_rt = wpool.tile([1, 64], F32R)
            nc.gpsimd.memset(onesf_t[:], 1.0)
            nc.vector.tensor_copy(ones_rt[:], onesf_t[:])
            ones_r = ones_rt[:]

            # persistent state: h group tiles (ping-pong on group parity)
            HDT = FP8E4 if FP8 else F32R
            h0g = [wpool.tile([128, 2, GRP, BL], HDT, tag=f"h0g{i}", name=f"h0g{i}")
                   for i in range(2)]
            h1g = [wpool.tile([128, 2, GRP, BL], HDT, tag=f"h1g{i}", name=f"h1g{i}")
                   for i in range(2)]
            h0r = [wpool.tile([128, 2, GRP, BL], F32R, tag=f"h0r{i}", name=f"h0r{i}")
                   for i in range(2)]
            h1r = [wpool.tile([128, 2, GRP, BL], F32R, tag=f"h1r{i}", name=f"h1r{i}")
                   for i in range(2)]
            zerof_t = wpool.tile([128, 2, GRP, BL], F32)
            nc.gpsimd.memset(zerof_t[:], 0.0)
            for t in (*h0g, *h1g, *h0r, *h1r):
                nc.vector.tensor_copy(t[:], zerof_t[:])
            gxt0 = [wpool.tile([128, 6, GRP, BL], F32R, tag=f"gxt0{i}", name=f"gxt0{i}")
                    for i in range(2)]
            gxt1 = [wpool.tile([128, 6, GRP, BL], F32R, tag=f"gxt1{i}", name=f"gxt1{i}")
                    for i in range(2)]
            xTa = [wpool.tile([I + 1, 64], F32R, tag=f"xTa{i}", name=f"xTa{i}")
                   for i in range(2)]
            yio = [wpool.tile([64, I], F32, tag=f"yio{i}", name=f"yio{i}")
                   for i in range(2)]
            zio = [wpool.tile([64, I], F32, tag=f"zio{i}", name=f"zio{i}")
                   for i in range(2)]
            yres = [wpool.tile([64, I], F32, tag=f"yres{i}", name=f"yres{i}")
                    for i in range(2)]
            for t in xTa:
                nc.vector.tensor_copy(t[I : I + 1, :], onesf_t[:])

            # PSUM gate banks: [128, 8mc, 8t, 8b] f32 = 2KB/partition
            psL0 = [ps_gate.tile([128, 8, GRP, BL], F32, tag=f"psL0{i}", name=f"psL0{i}")
                    for i in range(2)]
            psL1 = [ps_gate.tile([128, 8, GRP, BL], F32, tag=f"psL1{i}", name=f"psL1{i}")
                    for i in range(2)]

            def mm(out, lhsT, rhs, start, stop):
                nc.tensor.matmul(out, lhsT, rhs, start=start, stop=stop,
                                 skip_group_check=True)

            def gx0_bulk(par):
                """Compute gxT for L0 group (parity par) into PSUM, then
                copy to SBUF gxt0[par].  Bias row in w0T covers b_ih (+b_hh
                for rz chunks)."""
                P = psL0[par]
                xr = _r(xTa[par])
                for mc in range(6):
                    mm(P[:, mc, :, :], _r(w0T_t[:, mc, :]), xr, True, True)
                nc.vector.tensor_copy(gxt0[par][:], P[:, 0:6, :, :])

            def gx1_bulk(par):
                """Compute gxT for L1 group (parity par) from h0g[par]
                into PSUM, then copy to SBUF gxt1[par]."""
                P = psL1[par]
                hg = h0g[par]
                if FP8 and FP8_MIXED:
                    hgr = h0r[par]
                    for mc in range(4):
                        nc.tensor.matmul(P[:, mc, :, :], w1T_t[:, :, mc, :],
                                         hg[:], start=True, stop=True,
                                         perf_mode=DR, skip_group_check=True)
                    for c in range(2):
                        mm(P[:, 4 + c, :, :], _r(w1n_t[:, 0, c, :]),
                           hgr[:, 0, :, :], True, False)
                        mm(P[:, 4 + c, :, :], _r(w1n_t[:, 1, c, :]),
                           hgr[:, 1, :, :], False, True)
                elif FP8:
                    for mc in range(6):
                        nc.tensor.matmul(P[:, mc, :, :], w1T_t[:, :, mc, :],
                                         hg[:], start=True, stop=True,
                                         perf_mode=DR,
                                         skip_group_check=True)
                else:
                    hgk0 = hg[:, 0, :, :]
                    hgk1 = hg[:, 1, :, :]
                    for mc in range(6):
                        mm(P[:, mc, :, :], _r(w1T_t[:, 0, mc, :]), hgk0,
                           True, False)
                        mm(P[:, mc, :, :], _r(w1T_t[:, 1, mc, :]), hgk1,
                           False, True)
                nc.vector.scalar_tensor_tensor(gxt1[par][:], P[:, 0:6, :, :],
                                               0.0, b1bc_t[:], ADD, ADD)

            def step_mms(P, hT, k, whhT, gxT, bhn_t):
                """gh matmuls for one step, one closed accumulation
                group per chunk.  With RZ_FOLD_PE, rz chunks get gx
                folded via eye128 (PE); otherwise a DVE add does it.
                n chunks get b_hh_n folded via ones row."""
                if FP8 and FP8_MIXED:
                    hT8, hTr, whhnT, bnr = hT
                    for mc in (0, 1):
                        nc.tensor.matmul(P[:, mc, k, :], whhT[:, :, mc, :],
                                         hT8, start=True, stop=True,
                                         perf_mode=DR, skip_group_check=True)
                    for c in range(2):
                        mm(P[:, 4 + c, k, :], _r(whhnT[:, 0, c, :]),
                           hTr[:, 0, :], True, False)
                        mm(P[:, 4 + c, k, :], _r(whhnT[:, 1, c, :]),
                           hTr[:, 1, :], False, False)
                        mm(P[:, 4 + c, k, :], _r(bnr[:, c, :]),
                           ones_r[:, 0:BL], False, True)
                    for mc in (2, 3):
                        nc.tensor.matmul(P[:, mc, k, :], whhT[:, :, mc, :],
                                         hT8, start=True, stop=True,
                                         perf_mode=DR, skip_group_check=True)
                    return
                if FP8:
                    for mc in (0, 1, 4, 5, 2, 3):
                        nc.tensor.matmul(P[:, mc, k, :], whhT[:, :, mc, :],
                                         hT, start=True, stop=True,
                                         perf_mode=DR,
                                         skip_group_check=True)
                    return
                for mc in (0, 1):
                    mm(P[:, mc, k, :], _r(whhT[:, 0, mc, :]), hT[:, 0, :],
                       True, False)
                    mm(P[:, mc, k, :], _r(whhT[:, 1, mc, :]), hT[:, 1, :],
                       False, not RZ_FOLD_PE)
                    if RZ_FOLD_PE:
                        mm(P[:, mc, k, :], _r(eye128_t[:]), gxT[:, mc, k, :],
                           False, True)
                for mc in (4, 5):
                    mm(P[:, mc, k, :], _r(whhT[:, 0, mc, :]), hT[:, 0, :],
                       True, False)
                    mm(P[:, mc, k, :], _r(whhT[:, 1, mc, :]), hT[:, 1, :],
                       False, True)
                for mc in (2, 3):
                    mm(P[:, mc, k, :], _r(whhT[:, 0, mc, :]), hT[:, 0, :],
                       True, False)
                    mm(P[:, mc, k, :], _r(whhT[:, 1, mc, :]), hT[:, 1, :],
                       False, not RZ_FOLD_PE)
                    if RZ_FOLD_PE:
                        mm(P[:, mc, k, :], _r(eye128_t[:]), gxT[:, mc, k, :],
                           False, True)

            def lin_out(par, t0g, yr_t):  # yr_t = yres[par]
                """out(group) = h1g[par].T @ W_lin.T + b_lin + y, DMA out."""
                hg = h1g[par]
                f1 = ps_misc.tile([64, I], F32, tag="lin")
                if FP8 and FP8_MIXED:
                    hgr = h1r[par]
                    nc.tensor.matmul(f1, _r(hgr[:, 0, :, :]),
                                     _r(wlinT_t[:, 0, :]),
                                     start=True, stop=False)
                    nc.tensor.matmul(f1, _r(hgr[:, 1, :, :]),
                                     _r(wlinT_t[:, 1, :]),
                                     start=False, stop=True)
                elif FP8:
                    nc.tensor.matmul(f1, hg[:], wlinT_t[:], start=True,
                                     stop=True, perf_mode=DR,
                                     skip_group_check=True)
                else:
                    nc.tensor.matmul(f1, _r(hg[:, 0, :, :]),
                                     _r(wlinT_t[:, 0, :]),
                                     start=True, stop=False)
                    nc.tensor.matmul(f1, _r(hg[:, 1, :, :]),
                                     _r(wlinT_t[:, 1, :]),
                                     start=False, stop=True)
                o_t = opool.tile([64, I], F32, tag="o")
                nc.vector.tensor_tensor(o_t, f1, yr_t, ADD)
                nc.sync.dma_start(
                    out=out_d[:, ds(t0g, GRP), :].transpose([1, 0, 2]),
                    in_=o_t)

            def load_group(t0g, par):
                nc.sync.dma_start(
                    out=yio[par],
                    in_=y_d[:, ds(t0g, GRP), :].transpose([1, 0, 2]))
                nc.sync.dma_start(
                    out=zio[par],
                    in_=z_d[:, ds(t0g, GRP), :].transpose([1, 0, 2]))

            def load_yres(t0g, par):
                nc.sync.dma_start(
                    out=yres[par],
                    in_=y_d[:, ds(t0g, GRP), :].transpose([1, 0, 2]))
                nc.gpsimd.tensor_tensor(yres[par], yres[par], blinbc_t[:],
                                        ADD)

            def x_prep(par):
                x_t = iopool.tile([64, I], F32, tag="x")
                nc.vector.tensor_tensor(x_t, yio[par], zio[par], ADD)
                xp = ps_misc.tile([64, 64], F32, tag="tp")
                nc.tensor.transpose(xp, x_t, eye_t)
                nc.vector.tensor_copy(xTa[par][0:I, :], xp)

            def super_body(t0, s_par, do_l0, do_l1, has_next, has_next2):
                """One super-iteration. t0 = time offset of L0's group
                (int or symbolic); s_par = its parity."""
                p0 = s_par
                p1 = (s_par + 1) % 2  # parity of group s-1
                if do_l0:
                    P0, H0, Gx0 = psL0[p0], h0g[p0], gxt0[p0]
                    H0p = h0g[p1]
                    H0r, H0rp = h0r[p0], h0r[p1]
                if do_l1:
                    P1, H1, Gx1 = psL1[p1], h1g[p1], gxt1[p1]
                    H1p = h1g[p0]
                    H1r, H1rp = h1r[p1], h1r[p0]
                for k in range(GRP):
                    hp0 = hp1 = hp0r = hp1r = None
                    if do_l0:
                        hp0 = H0[:, :, k - 1, :] if k > 0 else H0p[:, :, 7, :]
                        hp0r = (H0r[:, :, k - 1, :] if k > 0
                                else H0rp[:, :, 7, :])
                        step_mms(P0, (hp0, hp0r, whh0n_t, bhh0nr_t)
                                 if FP8_MIXED else hp0, k, whh0T_t, Gx0,
                                 bhh0n_t)
                    if do_l1:
                        hp1 = H1[:, :, k - 1, :] if k > 0 else H1p[:, :, 7, :]
                        hp1r = (H1r[:, :, k - 1, :] if k > 0
                                else H1rp[:, :, 7, :])
                        step_mms(P1, (hp1, hp1r, whh1n_t, bhh1nr_t)
                                 if FP8_MIXED else hp1, k, whh1T_t, Gx1,
                                 bhh1n_t)

                    tl = {}
                    for ell, active in ((0, do_l0), (1, do_l1)):
                        if not active:
                            continue
                        for nm in ("m", "t", "n", "a", "p"):
                            tl[nm, ell] = gpool.tile(
                                [128, 2, BL], F32, tag=f"{nm}{ell}",
                                name=f"{nm}{ell}")
                        tl["rz", ell] = gpool.tile(
                            [128, 4, BL], F32, tag=f"rz{ell}",
                            name=f"rz{ell}")
                    # phase A: in-place gx add on PSUM (DVE), then sigmoids
                    for ell, active, P, Gx in (
                            (0, do_l0, psL0[p0] if do_l0 else None,
                             gxt0[p0] if do_l0 else None),
                            (1, do_l1, psL1[p1] if do_l1 else None,
                             gxt1[p1] if do_l1 else None)):
                        if not active:
                            continue
                        if Q_INPLACE:
                            nc.vector.tensor_tensor(
                                P[:, 0:4, k, :], P[:, 0:4, k, :],
                                Gx[:, 0:4, k, :].bitcast(F32), ADD)
                        else:
                            tl["q", ell] = gpool.tile(
                                [128, 4, BL], F32, tag=f"q{ell}",
                                name=f"q{ell}")
                            nc.vector.tensor_tensor(
                                tl["q", ell], P[:, 0:4, k, :],
                                Gx[:, 0:4, k, :].bitcast(F32), ADD)
                    for ell, active, P in ((0, do_l0, psL0[p0] if do_l0 else None),
                                           (1, do_l1, psL1[p1] if do_l1 else None)):
                        if not active:
                            continue
                        nc.scalar.activation(
                            tl["rz", ell],
                            P[:, 0:4, k, :] if Q_INPLACE else tl["q", ell],
                            SIG)
                    # phase B: m = r * ghn (DVE, both PSUM)
                    for ell, active, P in (
                            (0, do_l0, psL0[p0] if do_l0 else None),
                            (1, do_l1, psL1[p1] if do_l1 else None)):
                        if not active:
                            continue
                        nc.vector.tensor_tensor(
                            tl["m", ell], tl["rz", ell][:, 0:2, :],
                            P[:, 4:6, k, :], MULT)
                    if do_l0:
                        nc.vector.tensor_tensor(tl["t", 0], tl["m", 0],
                                                Gx0[:, 4:6, k, :].bitcast(F32),
                                                ADD)
                    if do_l1:
                        nc.vector.tensor_tensor(tl["t", 1], tl["m", 1],
                                                Gx1[:, 4:6, k, :].bitcast(F32),
                                                ADD)
                    # off-chain: a = z*h_prev (Pool)
                    if do_l0:
                        nc.gpsimd.tensor_tensor(
                            tl["a", 0], tl["rz", 0][:, 2:4, :],
                            hp0r.bitcast(F32) if FP8_MIXED
                            else (hp0 if FP8 else hp0.bitcast(F32)), MULT)
                    if do_l1:
                        nc.gpsimd.tensor_tensor(
                            tl["a", 1], tl["rz", 1][:, 2:4, :],
                            hp1r.bitcast(F32) if FP8_MIXED
                            else (hp1 if FP8 else hp1.bitcast(F32)), MULT)
                    # phase C: tanh (ACT)
                    if do_l0:
                        nc.scalar.activation(tl["n", 0], tl["t", 0], TANH)
                    if do_l1:
                        nc.scalar.activation(tl["n", 1], tl["t", 1], TANH)
                    # phase D: p2 = (z-1)*n (DVE stt), h' = a - p2
                    if do_l0:
                        nc.vector.scalar_tensor_tensor(
                            tl["p", 0], tl["rz", 0][:, 2:4, :], 1.0,
                            tl["n", 0], SUB, MULT)
                    if do_l1:
                        nc.vector.scalar_tensor_tensor(
                            tl["p", 1], tl["rz", 1][:, 2:4, :], 1.0,
                            tl["n", 1], SUB, MULT)
                    if do_l0:
                        nc.vector.tensor_tensor(H0[:, :, k, :], tl["a", 0],
                                                tl["p", 0], SUB)
                        if FP8_MIXED:
                            nc.gpsimd.tensor_tensor(H0r[:, :, k, :],
                                                    tl["a", 0], tl["p", 0],
                                                    SUB)
                    if do_l1:
                        nc.vector.tensor_tensor(H1[:, :, k, :], tl["a", 1],
                                                tl["p", 1], SUB)
                        if FP8_MIXED:
                            nc.gpsimd.tensor_tensor(H1r[:, :, k, :],
                                                    tl["a", 1], tl["p", 1],
                                                    SUB)

                # ---- bulk / io ----
                if do_l0:
                    gx1_bulk(p0)                      # feeds L1 next super
                if do_l1:
                    lin_out(p1, t0 - GRP, yres[p1])
                if has_next:
                    x_prep(p1)                        # (s+1) % 2 == p1
                    gx0_bulk(p1)
                if has_next2:
                    load_group(t0 + 2 * GRP, p0)      # (s+2) % 2 == p0
                if do_l0:
                    load_yres(t0, p0)

            def run_super(s):
                super_body(s * GRP, s % 2, s < SUP, s >= 1,
                           s + 1 < SUP, s + 2 < SUP)

            # ---------- prologue ----------
            load_group(0, 0)
            if SUP > 1:
                load_group(GRP, 1)
            x_prep(0)
            gx0_bulk(0)

            n_loop = SUP - 4
            for _rep in range(repeats):
                if _rep > 0:
                    load_group(0, 0)
                    if SUP > 1:
                        load_group(GRP, 1)
                    x_prep(0)
                    gx0_bulk(0)
                if not use_loop or n_loop <= 0 or n_loop % unroll != 0:
                    for s in range(0, SUP + 1):
                        run_super(s)
                else:
                    run_super(0)
                    run_super(1)
                    with tc.For_i(2 * GRP, (2 + n_loop) * GRP,
                                  GRP * unroll,
                                  staggered_reset=STAGGERED) as iv:
                        for j in range(unroll):
                            if j > 0 and unroll == 4 and STAGE_BOUNDARIES:
                                tc.stage_boundary()
                            super_body(iv + j * GRP, j % 2, True, True,
                                       True, True)
                    for s in range(SUP - 2, SUP + 1):
                        run_super(s)

            if debug:
                nc.sync.dma_start(out=dbg["gxn0"][:],
                                  in_=gxt0[0][:, 4:6, :, :].bitcast(F32))
                h0c = wpool.tile([128, 2, GRP, BL], F32)
                nc.vector.tensor_copy(h0c[:], h0g[0][:])
                nc.sync.dma_start(out=dbg["h0g"][:], in_=h0c[:])
                nc.sync.dma_start(out=dbg["gxn1"][:],
                                  in_=gxt1[0][:, 4:6, :, :].bitcast(F32))
                h1c = wpool.tile([128, 2, GRP, BL], F32)
                nc.vector.tensor_copy(h1c[:], h1g[0][:])
                nc.sync.dma_start(out=dbg["h1g"][:], in_=h1c[:])
                ps0_s = wpool.tile([128, 6, GRP, BL], F32)
                ps1_s = wpool.tile([128, 6, GRP, BL], F32)
                nc.vector.tensor_copy(ps0_s[:], psL0[0][:])
                nc.vector.tensor_copy(ps1_s[:], psL1[0][:])
                nc.sync.dma_start(out=dbg["ps0"][:], in_=ps0_s[:])
                nc.sync.dma_start(out=dbg["ps1"][:], in_=ps1_s[:])

    nc.compile()
    return nc


def prep_weights(W_ih0, W_hh0, b_ih0, b_hh0, W_ih1, W_hh1, b_ih1, b_hh1,
                 W_lin, b_lin):
    """Host-side weight folding into transposed-chunk layouts."""
    f = np.float32
    import ml_dtypes
    wdt = ml_dtypes.float8_e4m3fn if FP8 else f

    def whh_chunks(W):  # [768, K] -> [K..128, 2, 6, 128]
        Wt = np.ascontiguousarray(W.T).astype(f)  # [K, 768]
        K = Wt.shape[0]
        return np.ascontiguousarray(
            Wt.reshape(K // 128, 128, 6, 128).transpose(1, 0, 2, 3)
        ).astype(wdt)

    def whh_n_chunks(W):  # n-gate chunks only, f32r layout [128, 2, 2, 128]
        Wt = np.ascontiguousarray(W.T).astype(f)  # [K, 768]
        K = Wt.shape[0]
        full = Wt.reshape(K // 128, 128, 6, 128).transpose(1, 0, 2, 3)
        return np.ascontiguousarray(full[:, :, 4:6, :])

    # L0 input weights + bias row (rz: b_ih+b_hh, n: b_ih only)
    brow0 = np.concatenate([(b_ih0 + b_hh0)[: 2 * H], b_ih0[2 * H :]])
    w0T = np.concatenate([W_ih0.T, brow0[None, :]], axis=0).astype(f)
    w0T = np.ascontiguousarray(w0T.reshape(I + 1, 6, 128))

    return {
        "w0T": w0T,
        "whh0T": whh_chunks(W_hh0),
        "bhh0n": np.ascontiguousarray(
            b_hh0[2 * H :].reshape(2, 128).T).astype(f),
        "w1T": whh_chunks(W_ih1),
        "whh1T": whh_chunks(W_hh1),
        "b1bc": np.ascontiguousarray(np.broadcast_to(
            np.concatenate([(b_ih1 + b_hh1)[: 2 * H], b_ih1[2 * H :]]
                           ).reshape(6, 128).T[:, :, None, None],
            (128, 6, GRP, BL))).astype(f),
        "eye128": np.eye(128, dtype=f),
        "bhh1n": np.ascontiguousarray(
            b_hh1[2 * H :].reshape(2, 128).T).astype(f),
        "wlinT": np.ascontiguousarray(
            W_lin.T.astype(f).reshape(2, 128, I).transpose(1, 0, 2)
        ).astype(f if FP8_MIXED else wdt),
        "whh0n": whh_n_chunks(W_hh0),
        "bhh0nr": np.ascontiguousarray(
            b_hh0[2 * H :].reshape(1, 2, 128)).astype(f),
        "bhh1nr": np.ascontiguousarray(
            b_hh1[2 * H :].reshape(1, 2, 128)).astype(f),
        "whh1n": whh_n_chunks(W_hh1),
        "w1n": whh_n_chunks(W_ih1),
        "blinbc": np.ascontiguousarray(
            np.broadcast_to(b_lin[None, :], (64, I))).astype(f),
        "eye64": np.eye(64, dtype=f),
    }


_NC_CACHE = {}


def kernel(z, y, W_ih0, W_hh0, b_ih0, b_hh0, W_ih1, W_hh1, b_ih1, b_hh1,
           W_lin, b_lin, _trace=False):
    """Full-input entry point: shards over 8 cores, returns full output."""
    from concourse.bass_utils import run_bass_kernel_spmd

    z = np.asarray(z, np.float32)
    y = np.asarray(y, np.float32)
    T = z.shape[1]
    if T not in _NC_CACHE:
        _NC_CACHE[T] = build_nc(T=T)
    nc = _NC_CACHE[T]

    wmaps = prep_weights(
        np.asarray(W_ih0), np.asarray(W_hh0), np.asarray(b_ih0),
        np.asarray(b_hh0), np.asarray(W_ih1), np.asarray(W_hh1),
        np.asarray(b_ih1), np.asarray(b_hh1), np.asarray(W_lin),
        np.asarray(b_lin))
    in_maps = []
    for c in range(NCORES):
        sl = slice(c * BL, (c + 1) * BL)
        m = {"z": np.ascontiguousarray(z[sl]),
             "y": np.ascontiguousarray(y[sl])}
        m.update(wmaps)
        in_maps.append(m)

    res = run_bass_kernel_spmd(nc, in_maps, list(range(NCORES)), trace=_trace)
    outs = [res.results[c]["out"] for c in range(NCORES)]
    full = np.concatenate(outs, axis=0).astype(np.float32)
    if _trace:
        return full, res
    return full
